# revision 15
# baseline (speedup 1.0000x reference)
"""Causal self-attention on 8 TRN2 NeuronCores.

Sharding: rank r = 2*b + g  (b = batch 0..3, g = head-group 0..1; 8 heads/group).
Per core: QKV projection for its head-group, causal attention with scores kept
transposed — (tk, tq) tiles so the softmax denominator comes from a ones-column
folded into V (no cross-partition reduction; no max-subtraction needed at these
score magnitudes) — pairwise AllGather of the per-group attention outputs (one
collective per 512-wide tq block, overlapped with the remaining attention
blocks), then the output projection for this group's output columns. Host only
shards/transposes inputs and reassembles the output.

All matmuls run in float32r (full-rate 4-byte PE path, ~TF32 accuracy).
Diagonal score tiles are column-restricted: only columns >= 128*min(o4,2) are
computed/masked/contracted, so the causal mask shrinks to small additive
strips and invalid columns are never produced. The softmax division uses a
partition-broadcast of 1/Z via an SBUF->SBUF DMA, keeping PE out of the
normalize path.
"""
import numpy as np

import concourse.bass as bass
import concourse.mybir as mybir
import concourse.tile as tile
from concourse import bacc
from concourse.bass_utils import run_bass_kernel_spmd

F32 = mybir.dt.float32
F32R = mybir.dt.float32r
EXP = mybir.ActivationFunctionType.Exp

B, T, C, H, HD = 4, 2048, 1024, 16, 64
G, HG, CG = 2, 8, 512          # head groups, heads/group, channels/group
TH = T // 2
NCORES = 8
NEG = -1.0e30

_cache = {}


def _build(unroll=1, timeline=False):
    nc = bacc.Bacc("TRN2", target_bir_lowering=False, debug=False,
                   num_devices=NCORES)

    xT = nc.dram_tensor("xT", [C, T], F32R, kind="ExternalInput")
    w_qT = nc.dram_tensor("w_qT", [C, CG], F32R, kind="ExternalInput")
    w_kT = nc.dram_tensor("w_kT", [C, CG], F32R, kind="ExternalInput")
    w_vT = nc.dram_tensor("w_vT", [C, CG], F32R, kind="ExternalInput")
    w_oT = nc.dram_tensor("w_oT", [C, CG], F32R, kind="ExternalInput")
    maskadd = nc.dram_tensor("maskadd", [128, 128], F32, kind="ExternalInput")
    maskadd3 = nc.dram_tensor("maskadd3", [128, 256], F32, kind="ExternalInput")
    yT = nc.dram_tensor("yT", [CG, T], F32, kind="ExternalOutput")

    ag_in = [nc.dram_tensor(f"ag_in{i}", [4, 128, 512], F32R) for i in range(4)]
    ag_out = [nc.dram_tensor(f"ag_out{i}", [2, 4, 128, 512], F32R)
              for i in range(4)]

    with tile.TileContext(nc) as tc:
      for _it in range(unroll):
        with tc.tile_pool(name="attn_data", bufs=1) as p_data, \
             tc.tile_pool(name="consts", bufs=1) as p_const:
            qT = p_data.tile([128, 4, T], F32R, tag="qT")     # (ch%128, ch//128, t)
            kT = p_data.tile([128, 4, T], F32R, tag="kT")
            v_aug = p_data.tile([128, 16, HG, HD + 1], F32R, tag="v")  # (t%128, t//128, h, d|1)
            wo_all = p_data.tile([128, 8, CG], F32R, tag="wo")
            masks = p_const.tile([128, 128], F32, tag="masks")
            masks3 = p_const.tile([128, 256], F32, tag="masks3")

            xT_r = xT[:].rearrange("(ct p) t -> p ct t", p=128)       # (128, 8, T)
            wq_r = w_qT[:].rearrange("(ct p) m -> p ct m", p=128)     # (128, 8, CG)
            wk_r = w_kT[:].rearrange("(ct p) m -> p ct m", p=128)
            wv_r = w_vT[:].rearrange("(ct p) m -> p ct m", p=128)
            wo_r = w_oT[:].rearrange("(ct p) m -> p ct m", p=128)

            # ---- Phase 1: QKV projection (t-quarters, ping-pong x tiles;
            # K,Q first so the attention score matmuls can start early) ----
            with tc.tile_pool(name="xq", bufs=2) as p_x, \
                 tc.tile_pool(name="wqk", bufs=1) as p_w, \
                 tc.tile_pool(name="ps_qkv", bufs=2, space="PSUM") as p_ps:
                wk_all = p_w.tile([128, 8, CG], F32R, tag="wk_all")
                wq_all = p_w.tile([128, 8, CG], F32R, tag="wq_all")
                vstrip = p_w.tile([128, 8, CG], F32R, tag="vstrip")
                nc.sync.dma_start(out=wk_all[:], in_=wk_r)
                nc.sync.dma_start(out=masks[:], in_=maskadd[:])
                nc.sync.dma_start(out=masks3[:], in_=maskadd3[:])
                ones_f32 = p_const.tile([128, 1], F32, tag="ones_f32")
                nc.vector.memset(ones_f32[:], 1.0)
                nc.vector.tensor_copy(
                    out=v_aug[:, :, :, HD:HD + 1],
                    in_=ones_f32[:].to_broadcast([128, 16, HG, 1]))
                for tq in range(4):
                    t0 = tq * 512
                    xq = p_x.tile([128, 8, 512], F32R, tag="xq")
                    for ct in range(8):
                        nc.sync.dma_start(out=xq[:, ct, :],
                                          in_=xT_r[:, ct, t0:t0 + 512])
                    if tq == 0:
                        nc.sync.dma_start(out=wq_all[:], in_=wq_r)
                        nc.sync.dma_start(out=vstrip[:], in_=wv_r)
                    # K then Q: (ch, t) layout
                    for dest, wsrc in ((kT, wk_all), (qT, wq_all)):
                        for kt in range(4):
                            ps = p_ps.tile([128, 512], F32, tag="ps")
                            for ct in range(8):
                                nc.tensor.matmul(
                                    ps[:], wsrc[:, ct, kt * 128:(kt + 1) * 128],
                                    xq[:, ct, :],
                                    start=(ct == 0), stop=(ct == 7))
                            nc.vector.tensor_copy(
                                out=dest[:, kt, t0:t0 + 512], in_=ps[:])
                    # V: (t, ch) layout, interleaved with the ones column
                    for mm in range(4):
                        m = tq * 4 + mm
                        ps = p_ps.tile([128, 512], F32, tag="ps")
                        for ct in range(8):
                            nc.tensor.matmul(ps[:], xq[:, ct, mm * 128:(mm + 1) * 128],
                                             vstrip[:, ct, :],
                                             start=(ct == 0), stop=(ct == 7))
                        nc.vector.tensor_copy(
                            out=v_aug[:, m, :, 0:HD],
                            in_=ps[:].rearrange("p (h d) -> p h d", h=HG))

            # ---- Phase 2+3+4: attention; AllGather + out-proj per tq block
            # The (n, h, grp) group stream is software-pipelined: each group's
            # AV matmuls are emitted one group later, so PE (in-order) always
            # has the next group's score matmuls to chew while ACT runs exp.
            # Out-projection matmuls are spread through later blocks as
            # fillers; per-head normalize and per-block collectives are
            # emitted when their last AV retires.
            with tc.tile_pool(name="ps_s", bufs=2, space="PSUM") as p_s, \
                 tc.tile_pool(name="ps_o", bufs=3, space="PSUM") as p_o, \
                 tc.tile_pool(name="ps_y", bufs=1, space="PSUM") as p_y, \
                 tc.tile_pool(name="expS", bufs=5) as p_e, \
                 tc.tile_pool(name="small", bufs=2) as p_sm, \
                 tc.tile_pool(name="slabs", bufs=2) as p_sl, \
                 tc.tile_pool(name="ysb", bufs=2) as p_ysb:
                import collections as _c
                fillers = _c.deque()

                def _enqueue_outproj(n):
                    slab = p_sl.tile([128, 8, 512], F32R, tag="slab")
                    for ci in range(8):
                        src = (ag_out[n][ci // 4, ci % 4] if not timeline
                               else ag_in[n][ci % 4])
                        nc.sync.dma_start(out=slab[:, ci, :], in_=src)
                    for co in range(4):
                        y_ps = p_y.tile([128, 512], F32, tag="y")
                        y_sb = p_ysb.tile([128, 512], F32, tag="ysb")

                        def t1(co=co, y_ps=y_ps, slab=slab):
                            for ci in range(4):
                                nc.tensor.matmul(
                                    y_ps[:], wo_all[:, ci, co * 128:(co + 1) * 128],
                                    slab[:, ci, :],
                                    start=(ci == 0), stop=False)

                        def t2(n=n, co=co, y_ps=y_ps, y_sb=y_sb, slab=slab):
                            for ci in range(4, 8):
                                nc.tensor.matmul(
                                    y_ps[:], wo_all[:, ci, co * 128:(co + 1) * 128],
                                    slab[:, ci, :],
                                    start=False, stop=(ci == 7))
                            nc.vector.tensor_copy(out=y_sb[:], in_=y_ps[:])
                            nc.sync.dma_start(
                                out=yT[co * 128:(co + 1) * 128,
                                       n * 512:(n + 1) * 512],
                                in_=y_sb[:])
                        fillers.append(t1)
                        fillers.append(t2)

                groups = []
                for n in range(4):
                    for h in range(HG):
                        ngrp = 2 * n + 2
                        for grp in range(ngrp):
                            groups.append((n, h, grp, ngrp))

                o_ps_cur = {}

                def _retire(g):
                    n, h, grp, ngrp = g
                    kt, po = h // 2, (h % 2) * 64
                    o_ps, e_sb = o_ps_cur[(n, h)]["o"], o_ps_cur[(n, h)]["e"][grp]
                    ntiles = 2 * ngrp
                    for jj in range(2):
                        m = grp * 2 + jj
                        o4 = m - 4 * n
                        j0 = 0 if o4 < 0 else 128 * min(o4, 2)
                        nc.tensor.matmul(
                            o_ps[:, j0:512], v_aug[:, m, h, :],
                            e_sb[:, jj, j0:512],
                            start=(m == 0), stop=(m == ntiles - 1))
                    if grp == ngrp - 1:
                        # normalize: copy out of PSUM first (releases the
                        # accumulator bank early), then 1/Z partition-broadcast
                        # on GPSIMD and one DVE multiply
                        o_sb = p_sm.tile([HD + 1, 512], F32, tag="osb")
                        nc.vector.tensor_copy(out=o_sb[:], in_=o_ps[:])
                        rz = p_sm.tile([1, 512], F32, tag="rz")
                        nc.vector.reciprocal(rz[:], o_sb[HD:HD + 1, :])
                        rzb = p_sm.tile([HD, 512], F32, tag="rzb")
                        nc.gpsimd.partition_broadcast(rzb[:], rz[:])
                        onrm = p_sm.tile([64, 512], F32R, tag="onrm")
                        nc.vector.tensor_mul(onrm[:], o_sb[0:HD, :], rzb[:])
                        ag_in_r = ag_in[n][:].rearrange("a p t -> p a t")
                        nc.sync.dma_start(
                            out=ag_in_r[po:po + 64, kt, :], in_=onrm[:])
                        del o_ps_cur[(n, h)]
                        if h == HG - 1:
                            if not timeline:
                                nc.gpsimd.collective_compute(
                                    "AllGather", mybir.AluOpType.bypass,
                                    replica_groups=[[0, 1], [2, 3],
                                                    [4, 5], [6, 7]],
                                    ins=[ag_in[n][:]], outs=[ag_out[n][:]])
                            _enqueue_outproj(n)

                nc.sync.dma_start(out=wo_all[:], in_=wo_r)
                pend = _c.deque()
                for gi, g in enumerate(groups):
                    n, h, grp, ngrp = g
                    kt, po = h // 2, (h % 2) * 64
                    if grp == 0:
                        o_ps_cur[(n, h)] = {
                            "o": p_o.tile([HD + 1, 512], F32, tag="o",
                                          name=f"ops_{n}_{h}"),
                            "e": {}}
                    s_ps = p_s.tile([128, 2, 512], F32, tag="s")
                    e_sb = p_e.tile([128, 2, 512], F32R, tag="e")
                    o_ps_cur[(n, h)]["e"][grp] = e_sb
                    for jj in range(2):
                        m = grp * 2 + jj
                        o4 = m - 4 * n
                        # column restriction: o4=3 keeps N=256 so the f32r
                        # matmul stays in its fast regime
                        j0 = 0 if o4 < 0 else 128 * min(o4, 2)
                        nc.tensor.matmul(
                            s_ps[:, jj, j0:512],
                            kT[po:po + 64, kt, m * 128:(m + 1) * 128],
                            qT[po:po + 64, kt, n * 512 + j0:(n + 1) * 512],
                            start=True, stop=True)
                        if 0 <= o4 <= 2:
                            nc.vector.tensor_add(
                                s_ps[:, jj, 128 * o4:128 * o4 + 128],
                                s_ps[:, jj, 128 * o4:128 * o4 + 128],
                                masks[:])
                        elif o4 == 3:
                            nc.vector.tensor_add(
                                s_ps[:, jj, 256:512],
                                s_ps[:, jj, 256:512],
                                masks3[:])
                    if grp == ngrp - 1 and n > 0:
                        # last group holds the o4=2,3 diagonal tiles; only
                        # columns >= 256 are ever contracted
                        nc.scalar.activation(e_sb[:, :, 256:512],
                                             s_ps[:, :, 256:512], EXP,
                                             scale=0.125)
                    else:
                        nc.scalar.activation(e_sb[:], s_ps[:], EXP, scale=0.125)
                    pend.append(g)
                    if len(pend) > 3:
                        _retire(pend.popleft())
                    if fillers and gi % 3 == 0:
                        fillers.popleft()()
                while pend:
                    _retire(pend.popleft())
                while fillers:
                    fillers.popleft()()

    nc.compile()
    return nc


def _mask_np():
    i = np.arange(128, dtype=np.int64)[:, None]
    j = np.arange(128, dtype=np.int64)[None, :]
    return np.where(i > j, np.float32(NEG), np.float32(0.0)).astype(np.float32)


def _mask3_np():
    # o4 = 3 diagonal tile, columns [256, 512): global col j = 256 + jloc,
    # invalid iff 384 + i > j  <=>  i > jloc - 128
    i = np.arange(128, dtype=np.int64)[:, None]
    jloc = np.arange(256, dtype=np.int64)[None, :]
    return np.where(i > jloc - 128, np.float32(NEG),
                    np.float32(0.0)).astype(np.float32)


def _in_maps(x, w_qkv, w_out):
    mask = _mask_np()
    mask3 = _mask3_np()
    maps = []
    for r in range(NCORES):
        b, g = r // 2, r % 2
        maps.append({
            "xT": np.ascontiguousarray(x[b].T),
            "w_qT": np.ascontiguousarray(w_qkv[g * CG:(g + 1) * CG, :].T),
            "w_kT": np.ascontiguousarray(w_qkv[C + g * CG:C + (g + 1) * CG, :].T),
            "w_vT": np.ascontiguousarray(w_qkv[2 * C + g * CG:2 * C + (g + 1) * CG, :].T),
            "w_oT": np.ascontiguousarray(w_out[g * CG:(g + 1) * CG, :].T),
            "maskadd": mask,
            "maskadd3": mask3,
        })
    return maps


def _run(x, w_qkv, w_out, trace=False):
    if "nc" not in _cache:
        _cache["nc"] = _build()
    res = run_bass_kernel_spmd(_cache["nc"], _in_maps(x, w_qkv, w_out),
                               list(range(NCORES)), trace=trace)
    y = np.empty((B, T, C), np.float32)
    for r in range(NCORES):
        b, g = r // 2, r % 2
        y[b, :, g * CG:(g + 1) * CG] = res.results[r]["yT"].T
    return y, res


def kernel(x, w_qkv, w_out):
    x = np.asarray(x, dtype=np.float32)
    w_qkv = np.asarray(w_qkv, dtype=np.float32)
    w_out = np.asarray(w_out, dtype=np.float32)
    y, _ = _run(x, w_qkv, w_out)
    return y
